# revision 39
# baseline (speedup 1.0000x reference)
"""BidLatte (linear-attention) Trainium2 kernel, 8-core SPMD.

Math (per batch b):
  K = X@Wk; Q = X@Wq; E = exp(K)*mask          (max-shift cancels exactly)
  Ksum = sum_t E;  KX = E^T @ X                (L x D state, avoids X@Wv)
  KXn = KX / Ksum; Kv = KXn @ Wv; Kv_bd = blockdiag_head(Kv)
  G = Kv_bd @ o_proj                           (o_proj folded into state)
  out = softmax_head(Q) @ G

Sharding: core 2i+j -> batch i, T-half j.

Schedule:
  A-K:   K-side state accumulation (DMA-bound).  X natural and X^T
         blocks alternate between the sync HWDGE ring and the gpsimd
         SWDGE ring so each block's pair lands together; X^T blocks
         build a resident SBUF tile reused by the Q side.  Q-side
         projections for blocks 0-3 fill the PE's DMA-starvation holes.
         The state is exchanged as TWO partial bf16 AllToAlls (shard =
         head half, per-rank shard selection is automatic): #1 covers
         blocks 0-5 and its own ncfw cold start under the A-K tail,
         #2 (blocks 6-7) hides under the post-A-K Q work.
  A-Q:   head-softmax of blocks 0-3 + projection/softmax of 4-5.
  B:     partial states summed on DVE (fp32), then transpose-free:
         the half-state returns as KX^T chunks via an xbar
         DMA-transpose, Kv is computed transposed (lhsT = Wv 128-col
         slices), the blockdiag mask applies in transposed form, and
         1/Ksum folds into the final G scale.  Each core computes only
         its 8-head half of G (only half of Wv / o_proj is DMA'd).
  A-Q2:  blocks 6-7 cover the small bf16 AllGather of G.
  C:     out = softmax(Q)^T^T @ G, stored bf16 (host converts to f32),
         stores alternating between the sync and gpsimd rings.

All matmuls bf16 (full PE rate); elementwise/exp/reductions and PSUM
accumulation fp32. Measured rel err vs fp32 reference ~6e-3.
"""
import numpy as np

_B, _T, _D, _L, _H = 4, 8192, 1024, 128, 16
NCORES = 8
TLOC = _T // 2  # tokens per core
BT = 512        # tokens per block
NBLK = TLOC // BT
NT = BT // 128  # t-tiles per block
DC = _D // 128  # d-chunks
NEARLY = 5      # Q blocks interleaved into A-K

_cache = {}


def _build():
    import concourse.bacc as bacc
    import concourse.mybir as mybir
    import concourse.tile as tile

    FP32 = mybir.dt.float32
    BF16 = mybir.dt.bfloat16
    EXP = mybir.ActivationFunctionType.Exp

    nc = bacc.Bacc("TRN2", target_bir_lowering=False, debug=False,
                   num_devices=NCORES)

    xs = nc.dram_tensor("xs", [TLOC, _D], BF16, kind="ExternalInput")
    xst = nc.dram_tensor("xst", [_D, TLOC], BF16, kind="ExternalInput")
    ms = nc.dram_tensor("ms", [128, TLOC // 128], FP32, kind="ExternalInput")
    wk = nc.dram_tensor("wk", [128, _D], BF16, kind="ExternalInput")
    wq = nc.dram_tensor("wq", [128, _D], BF16, kind="ExternalInput")
    wv = nc.dram_tensor("wv", [128, DC * 512], BF16, kind="ExternalInput")
    op = nc.dram_tensor("op", [128, 4 * _D], BF16, kind="ExternalInput")
    ident = nc.dram_tensor("ident", [128, 128], BF16, kind="ExternalInput")
    ph = nc.dram_tensor("ph", [128, _H], BF16, kind="ExternalInput")
    pht = nc.dram_tensor("pht", [_H, 128], BF16, kind="ExternalInput")
    ones2 = nc.dram_tensor("ones2", [128, 2], BF16, kind="ExternalInput")
    bdmt = nc.dram_tensor("bdmt", [128, 256], FP32, kind="ExternalInput")
    out = nc.dram_tensor("out", [TLOC, _D], BF16, kind="ExternalOutput")

    RG = [[0, 1], [2, 3], [4, 5], [6, 7]]

    with tile.TileContext(nc) as tc:
        with (
            tc.tile_pool(name="const", bufs=1) as cpool,
            tc.tile_pool(name="dram", bufs=1, space="DRAM") as dpool,
        ):
            xst_sb = cpool.tile([128, DC * TLOC], BF16)  # resident X^T
            qst_sb = cpool.tile([128, TLOC], BF16)       # softmax(Q)^T
            eqe_sb = cpool.tile([128, NEARLY * BT], BF16)  # early exp(Q)
            wk_sb = cpool.tile([128, _D], BF16)
            wq_sb = cpool.tile([128, _D], BF16)
            wv_sb = cpool.tile([128, DC * 512], BF16)
            op_sb = cpool.tile([128, 4 * _D], BF16)
            id_sb = cpool.tile([128, 128], BF16)
            ph_sb = cpool.tile([128, _H], BF16)
            pht_sb = cpool.tile([_H, 128], BF16)
            on_sb = cpool.tile([128, 2], BF16)
            ms_sb = cpool.tile([128, TLOC // 128], FP32)
            bdmt_sb = cpool.tile([128, 256], FP32)
            kxp_sb = cpool.tile([128, 1032], BF16)   # packed KX | Ksum
            p1_sb = cpool.tile([64, 1032], BF16)     # reduced partial 1
            p2_sb = cpool.tile([64, 1032], BF16)     # reduced partial 2
            shb_sb = cpool.tile([64, 1032], BF16)    # reduced half state
            sht_sb = cpool.tile([128, DC * 64], BF16)  # KX^T half chunks
            g_sb = cpool.tile([128, _D], BF16)       # gathered G

            aa1_in = dpool.tile([128, 1032], BF16)
            aa1_out = dpool.tile([64, 1032], BF16)
            aa2_in = dpool.tile([128, 1032], BF16)
            aa2_out = dpool.tile([64, 1032], BF16)
            ag_in = dpool.tile([64, _D], BF16)
            ag_out = dpool.tile([128, _D], BF16)
            wdum_in = dpool.tile([128, 2], BF16)
            wdum_out = dpool.tile([256, 2], BF16)

            # ------------- Phase A-K: KX / Ksum state accumulation ---------
            with (
                tc.tile_pool(name="xin", bufs=4) as xin,
                tc.tile_pool(name="esb", bufs=3) as esb,
                tc.tile_pool(name="e2", bufs=8) as e2p,
                tc.tile_pool(name="kt_ps", bufs=2, space="PSUM") as ktp,
                tc.tile_pool(name="scr_ps", bufs=2, space="PSUM") as scr,
                tc.tile_pool(name="kx_ps", bufs=1, space="PSUM") as kxp,
                tc.tile_pool(name="ks_ps", bufs=1, space="PSUM") as ksp,
            ):
                kx_ps = kxp.tile([128, _D], FP32)
                ks_ps = ksp.tile([128, 2], FP32)

                def ktx(k, xts, et):
                    """E-transpose + KX/KS accumulation for block k."""
                    e_ps = scr.tile([128, BT], BF16, tag="scr")
                    for i in range(NT):
                        nc.tensor.transpose(
                            e_ps[:, i * 128:(i + 1) * 128],
                            et[:, i * 128:(i + 1) * 128],
                            id_sb[:],
                        )
                    for i in range(NT):
                        e2 = e2p.tile([128, 128], BF16, tag="e2")
                        j = k * NT + i
                        nc.vector.tensor_scalar_mul(
                            e2[:], e_ps[:, i * 128:(i + 1) * 128],
                            ms_sb[:, j:j + 1],
                        )
                        first = (i == 0 and k == 0)
                        last = (i == NT - 1 and k == NBLK - 1)
                        nc.tensor.matmul(kx_ps[:, 0:512], e2[:],
                                         xts[i][:, 0:512],
                                         start=first, stop=last)
                        nc.tensor.matmul(kx_ps[:, 512:1024], e2[:],
                                         xts[i][:, 512:1024],
                                         start=first, stop=last)
                        nc.tensor.matmul(ks_ps[:], e2[:], on_sb[:],
                                         start=first, stop=last)

                def pack_state():
                    nc.vector.tensor_copy(kxp_sb[:, 0:512], kx_ps[:, 0:512])
                    nc.scalar.copy(kxp_sb[:, 512:1024], kx_ps[:, 512:1024])
                    nc.vector.tensor_copy(kxp_sb[:, 1024:1025],
                                          ks_ps[:, 0:1])
                    nc.vector.memset(kxp_sb[:, 1025:1032], 0.0)

                def load_x(k, xblk):
                    # alternate the two X streams across the two DMA rings
                    # so each block's pair arrives together
                    xb0 = k * DC * BT
                    xe = nc.gpsimd if k % 2 == 0 else nc.sync
                    xo = nc.sync if k % 2 == 0 else nc.gpsimd
                    xe.dma_start(
                        out=xblk[:].rearrange("p (a d) -> p a d", a=NT),
                        in_=xs.ap()[k * BT:(k + 1) * BT, :].rearrange(
                            "(a p) d -> p a d", p=128))
                    xo.dma_start(
                        out=xst_sb[:, xb0:xb0 + DC * BT].rearrange(
                            "p (c t) -> p c t", c=DC),
                        in_=xst.ap().rearrange("(c p) t -> p c t", p=128)
                        [:, :, k * BT:(k + 1) * BT])

                prev = None
                xblk7 = None
                for k in range(NBLK):
                    xb0 = k * DC * BT
                    if k == 0:
                        nc.sync.dma_start(out=wk_sb[:], in_=wk.ap())
                        nc.gpsimd.dma_start(out=id_sb[:], in_=ident.ap())
                        nc.gpsimd.dma_start(out=on_sb[:], in_=ones2.ap())
                        nc.gpsimd.dma_start(out=ms_sb[:], in_=ms.ap())
                        # tiny warm-up collective: absorbs the ncfw
                        # cold-start so RS#1 starts fast
                        nc.scalar.dma_start(out=wdum_in[:], in_=ones2.ap())
                        nc.gpsimd.collective_compute(
                            "AllGather",
                            mybir.AluOpType.bypass,
                            replica_groups=RG,
                            ins=[wdum_in.opt()],
                            outs=[wdum_out.opt()],
                        )
                    if k == 1:
                        nc.sync.dma_start(out=wq_sb[:], in_=wq.ap())

                    if k == NBLK - 1:
                        xblk = xblk7  # prefetched at k == 6
                    else:
                        xblk = xin.tile([128, NT * _D], BF16, tag="xin")
                        load_x(k, xblk)
                    xts = [xblk[:, i * _D:(i + 1) * _D] for i in range(NT)]
                    if k == 6:
                        # prefetch block 7's X so nothing can block it in
                        # the ring FIFOs later
                        xblk7 = xin.tile([128, NT * _D], BF16, tag="xin")
                        load_x(NBLK - 1, xblk7)

                    # ready work FIRST in the PE FIFO: interleaved Q-side
                    # projections (resident X^T) and the deferred previous
                    # ktx, so a kt waiting on this block's DMA cannot
                    # starve the PE into a HAM re-throttle
                    if 2 <= k <= 1 + NEARLY:
                        jq = k - 2
                        jb0 = jq * DC * BT
                        qt_ps = ktp.tile([128, BT], FP32, tag="kt")
                        for c in range(DC):
                            nc.tensor.matmul(
                                qt_ps[:], wq_sb[:, c * 128:(c + 1) * 128],
                                xst_sb[:, jb0 + c * BT:jb0 + (c + 1) * BT],
                                start=(c == 0), stop=(c == DC - 1),
                            )
                        nc.scalar.activation(
                            eqe_sb[:, jq * BT:(jq + 1) * BT], qt_ps[:], EXP)
                    if prev is not None:
                        ktx(*prev)

                    kt_ps = ktp.tile([128, BT], FP32, tag="kt")
                    for c in range(DC):
                        nc.tensor.matmul(
                            kt_ps[:], wk_sb[:, c * 128:(c + 1) * 128],
                            xst_sb[:, xb0 + c * BT:xb0 + (c + 1) * BT],
                            start=(c == 0), stop=(c == DC - 1),
                        )
                    et = esb.tile([128, BT], BF16, tag="et")
                    nc.scalar.activation(et[:], kt_ps[:], EXP)
                    prev = (k, xts, et)
                ktx(*prev)

                # weights for phases B/C load after the X^T stream
                nc.gpsimd.dma_start(out=ph_sb[:], in_=ph.ap())
                nc.gpsimd.dma_start(out=pht_sb[:], in_=pht.ap())
                nc.gpsimd.dma_start(out=bdmt_sb[:], in_=bdmt.ap())
                for c2 in range(DC):
                    nc.gpsimd.dma_start(
                        out=wv_sb[:, c2 * 512:(c2 + 1) * 512],
                        in_=wv.ap()[:, c2 * 512:(c2 + 1) * 512])
                for c2 in range(4):
                    nc.gpsimd.dma_start(
                        out=op_sb[:, c2 * _D:(c2 + 1) * _D],
                        in_=op.ap()[:, c2 * _D:(c2 + 1) * _D])

                pack_state()

            nc.sync.dma_start(out=aa2_in[:], in_=kxp_sb[:])
            nc.gpsimd.collective_compute(
                "ReduceScatter",
                mybir.AluOpType.add,
                replica_groups=RG,
                ins=[aa2_in.opt()],
                outs=[aa2_out.opt()],
            )
            nc.sync.dma_start(out=p2_sb[:], in_=aa2_out[:])

            # ------------- Phase A-Q + B + A-Q2 ---------------------------
            with (
                tc.tile_pool(name="eqsb", bufs=3) as eqsb,
                tc.tile_pool(name="srp", bufs=2) as srp,
                tc.tile_pool(name="bsb1", bufs=1) as bsb1,
                tc.tile_pool(name="qt_ps", bufs=2, space="PSUM") as qtp,
                tc.tile_pool(name="scr2_ps", bufs=3, space="PSUM") as scr2,
                tc.tile_pool(name="kvt_ps", bufs=1, space="PSUM") as kvtp,
                tc.tile_pool(name="g_ps_pool", bufs=1, space="PSUM") as gpp,
            ):
                def qsoft(k, eq):
                    s_ps = scr2.tile([_H, BT], FP32, tag="scr")
                    nc.tensor.matmul(s_ps[:], ph_sb[:], eq[:], start=True,
                                     stop=True)
                    sr = srp.tile([_H, BT], FP32, tag="sr")
                    nc.vector.reciprocal_approx_fast(sr[:], s_ps[:])
                    srb = srp.tile([_H, BT], BF16, tag="srb")
                    nc.vector.tensor_copy(srb[:], sr[:])
                    bq_ps = scr2.tile([128, BT], FP32, tag="scr")
                    nc.tensor.matmul(bq_ps[:], pht_sb[:], srb[:], start=True,
                                     stop=True)
                    nc.vector.tensor_mul(
                        qst_sb[:, k * BT:(k + 1) * BT], eq[:], bq_ps[:]
                    )

                def qphase(blocks):
                    prev = None
                    for k in blocks:
                        if k < NEARLY:
                            eq = eqe_sb[:, k * BT:(k + 1) * BT]
                        else:
                            qt_ps = qtp.tile([128, BT], FP32, tag="qt")
                            xb0 = k * DC * BT
                            for c in range(DC):
                                nc.tensor.matmul(
                                    qt_ps[:],
                                    wq_sb[:, c * 128:(c + 1) * 128],
                                    xst_sb[:, xb0 + c * BT:
                                           xb0 + (c + 1) * BT],
                                    start=(c == 0), stop=(c == DC - 1),
                                )
                            eqt = eqsb.tile([128, BT], BF16, tag="eq")
                            nc.scalar.activation(eqt[:], qt_ps[:], EXP)
                            eq = eqt[:]
                        if prev is not None:
                            qsoft(*prev)
                        prev = (k, eq)
                    qsoft(*prev)

                qphase(range(NBLK - 2))   # blocks 0-5 cover the A2As

                # ---- reduced half state ----------------------------------
                shb_sb = p2_sb
                ksf = bsb1.tile([64, 1], FP32)
                nc.vector.tensor_copy(ksf[:], p2_sb[:, 1024:1025])
                rks = bsb1.tile([64, 1], FP32)
                nc.vector.reciprocal_approx_fast(rks[:], ksf[:])
                # reduced half state as KX^T chunks via xbar transpose
                nc.sync.dma_start_transpose(
                    sht_sb[:].rearrange("p (c l) -> p c l", c=DC),
                    shb_sb[:, 0:1024])

                # ---- Phase B (half, transpose-free) -----------------------
                kvt_ps = kvtp.tile([128, 256], FP32)
                for e in range(4):
                    for c in range(DC):
                        nc.tensor.matmul(
                            kvt_ps[:, e * 64:(e + 1) * 64],
                            wv_sb[:, c * 512 + e * 128:
                                  c * 512 + (e + 1) * 128],
                            sht_sb[:, c * 64:(c + 1) * 64],
                            start=(c == 0), stop=(c == DC - 1),
                        )
                kvbdt = bsb1.tile([128, 256], BF16)
                nc.vector.tensor_mul(kvbdt[:], kvt_ps[:], bdmt_sb[:])

                g_half = bsb1.tile([64, _D], BF16)
                g_ps = gpp.tile([64, 512], FP32, tag="g")
                for e in range(4):
                    nc.tensor.matmul(
                        g_ps[:], kvbdt[:, e * 64:(e + 1) * 64],
                        op_sb[:, e * _D:e * _D + 512],
                        start=(e == 0), stop=(e == 3))
                nc.vector.tensor_scalar_mul(g_half[:, 0:512], g_ps[:],
                                            rks[:])
                g_ps2 = gpp.tile([64, 512], FP32, tag="g")
                for e in range(4):
                    nc.tensor.matmul(
                        g_ps2[:], kvbdt[:, e * 64:(e + 1) * 64],
                        op_sb[:, e * _D + 512:(e + 1) * _D],
                        start=(e == 0), stop=(e == 3))
                nc.vector.tensor_scalar_mul(g_half[:, 512:1024], g_ps2[:],
                                            rks[:])

                # ---- AllGather the two G halves ---------------------------
                nc.sync.dma_start(out=ag_in[:], in_=g_half[:])
                nc.gpsimd.collective_compute(
                    "AllGather",
                    mybir.AluOpType.bypass,
                    replica_groups=RG,
                    ins=[ag_in.opt()],
                    outs=[ag_out.opt()],
                )
                nc.sync.dma_start(out=g_sb[:], in_=ag_out[:])

                # Q blocks 6-7 cover the AllGather latency
                qphase(range(NBLK - 2, NBLK))

            # ------------- Phase C: out = Qs @ G --------------------------
            with (
                tc.tile_pool(name="osb", bufs=3) as osb,
                tc.tile_pool(name="ops", bufs=3, space="PSUM") as ops,
            ):
                for j in range(TLOC // 512):
                    ot = osb.tile([128, 4 * _D], BF16, tag="osb")
                    for h2 in range(4):
                        i = 4 * j + h2
                        o_ps = ops.tile([128, _D], FP32, tag="ops")
                        lhs = qst_sb[:, i * 128:(i + 1) * 128]
                        nc.tensor.matmul(o_ps[:, 0:512], lhs, g_sb[:, 0:512],
                                         start=True, stop=True)
                        nc.tensor.matmul(o_ps[:, 512:1024], lhs,
                                         g_sb[:, 512:1024], start=True,
                                         stop=True)
                        dst = ot[:, h2 * _D:(h2 + 1) * _D]
                        if i % 2 == 0:
                            nc.vector.tensor_copy(dst, o_ps[:])
                        else:
                            nc.scalar.copy(dst, o_ps[:])
                    # alternate output rings (gpsimd is idle in phase C)
                    eng = nc.sync if j % 2 == 0 else nc.gpsimd
                    eng.dma_start(
                        out=out.ap()[j * 512:(j + 1) * 512, :].rearrange(
                            "(a p) d -> p a d", p=128),
                        in_=ot[:].rearrange("p (a d) -> p a d", a=4))

    nc.compile()
    return nc


def _host_inputs(X, attention_mask, Wk, Wq, Wv, o_proj):
    import ml_dtypes

    BF = ml_dtypes.bfloat16
    X = np.asarray(X, dtype=np.float32)
    mask = np.asarray(attention_mask, dtype=np.float32)
    Wk = np.asarray(Wk, dtype=np.float32)
    Wq = np.asarray(Wq, dtype=np.float32)
    Wv = np.asarray(Wv, dtype=np.float32)
    o_proj = np.asarray(o_proj, dtype=np.float32)

    wk_r = np.ascontiguousarray(
        Wk.reshape(DC, 128, _L).transpose(1, 0, 2).reshape(128, DC * _L)
    ).astype(BF)
    wq_r = np.ascontiguousarray(
        Wq.reshape(DC, 128, _L).transpose(1, 0, 2).reshape(128, DC * _L)
    ).astype(BF)
    # per head-half slices of Wv (columns) and o_proj (rows)
    wv_half = []
    op_half = []
    for half in range(2):
        wvh = Wv[:, half * 512:(half + 1) * 512]           # (1024, 512)
        wv_half.append(np.ascontiguousarray(
            wvh.reshape(DC, 128, 512).transpose(1, 0, 2).reshape(
                128, DC * 512)).astype(BF))
        oph = o_proj[half * 512:(half + 1) * 512, :]       # (512, 1024)
        op_half.append(np.ascontiguousarray(
            oph.reshape(4, 128, _D).transpose(1, 0, 2).reshape(
                128, 4 * _D)).astype(BF))
    ident = np.eye(128, dtype=BF)
    ph_m = np.zeros((128, _H), dtype=BF)
    for hh in range(_H):
        ph_m[hh * (_L // _H):(hh + 1) * (_L // _H), hh] = 1.0
    pht_m = np.ascontiguousarray(ph_m.T)
    ones2 = np.ones((128, 2), dtype=BF)
    # transposed block-diag mask: bdmt[p, e*64+l] = 1 iff local d'-row
    # (e*128+p) belongs to the head of local column l
    bdmt_m = np.zeros((128, 256), dtype=np.float32)
    for e in range(4):
        for p in range(128):
            hh = (e * 128 + p) // 64
            bdmt_m[p, e * 64 + hh * 8:e * 64 + (hh + 1) * 8] = 1.0

    Xbf = X.astype(BF)
    in_maps = []
    for core in range(NCORES):
        b, half = core // 2, core % 2
        xsh = np.ascontiguousarray(Xbf[b, half * TLOC:(half + 1) * TLOC, :])
        xsth = np.ascontiguousarray(xsh.T)
        msh = np.ascontiguousarray(
            mask[b, half * TLOC:(half + 1) * TLOC]
            .reshape(TLOC // 128, 128).T)
        in_maps.append({
            "xs": xsh, "xst": xsth, "ms": msh, "wk": wk_r, "wq": wq_r,
            "wv": wv_half[half], "op": op_half[half], "ident": ident,
            "ph": ph_m, "pht": pht_m, "ones2": ones2, "bdmt": bdmt_m,
        })
    return in_maps


def _run(in_maps, trace=False):
    from concourse.bass_utils import run_bass_kernel_spmd

    if "nc" not in _cache:
        _cache["nc"] = _build()
    return run_bass_kernel_spmd(
        _cache["nc"], in_maps, list(range(NCORES)), trace=trace)


def kernel(X, attention_mask, Wk, Wq, Wv, o_proj, n_heads=16):
    in_maps = _host_inputs(X, attention_mask, Wk, Wq, Wv, o_proj)
    res = _run(in_maps)
    out = np.empty((_B, _T, _D), dtype=np.float32)
    for core in range(NCORES):
        b, half = core // 2, core % 2
        out[b, half * TLOC:(half + 1) * TLOC, :] = (
            res.results[core]["out"].astype(np.float32))
    return out


# revision 42
# speedup vs baseline: 1.0977x; 1.0977x over previous
"""BidLatte (linear-attention) Trainium2 kernel, 8-core SPMD.

Math (per batch b):
  K = X@Wk; Q = X@Wq; E = exp(K)*mask          (max-shift cancels exactly)
  Ksum = sum_t E;  KX = E^T @ X                (L x D state, avoids X@Wv)
  KXn = KX / Ksum; Kv = KXn @ Wv; Kv_bd = blockdiag_head(Kv)
  G = Kv_bd @ o_proj                           (o_proj folded into state)
  out = softmax_head(Q) @ G

Sharding: core 2i+j -> batch i, T-half j.

Schedule:
  A-K:   K-side state accumulation (DMA-bound).  X natural and X^T
         blocks alternate between the sync HWDGE ring and the gpsimd
         SWDGE ring so each block's pair lands together; X^T blocks
         build a resident SBUF tile reused by the Q side.  Q-side
         projections for blocks 0-3 fill the PE's DMA-starvation holes.
         The state is exchanged as TWO partial bf16 AllToAlls (shard =
         head half, per-rank shard selection is automatic): #1 covers
         blocks 0-5 and its own ncfw cold start under the A-K tail,
         #2 (blocks 6-7) hides under the post-A-K Q work.
  A-Q:   head-softmax of blocks 0-3 + projection/softmax of 4-5.
  B:     partial states summed on DVE (fp32), then transpose-free:
         the half-state returns as KX^T chunks via an xbar
         DMA-transpose, Kv is computed transposed (lhsT = Wv 128-col
         slices), the blockdiag mask applies in transposed form, and
         1/Ksum folds into the final G scale.  Each core computes only
         its 8-head half of G (only half of Wv / o_proj is DMA'd).
  A-Q2:  blocks 6-7 cover the small bf16 AllGather of G.
  C:     out = softmax(Q)^T^T @ G, stored bf16 (host converts to f32),
         stores alternating between the sync and gpsimd rings.

All matmuls bf16 (full PE rate); elementwise/exp/reductions and PSUM
accumulation fp32. Measured rel err vs fp32 reference ~6e-3.
"""
import numpy as np

_B, _T, _D, _L, _H = 4, 8192, 1024, 128, 16
NCORES = 8
TLOC = _T // 2  # tokens per core
BT = 512        # tokens per block
NBLK = TLOC // BT
NT = BT // 128  # t-tiles per block
DC = _D // 128  # d-chunks
NEARLY = 4      # Q blocks interleaved into A-K

_cache = {}


def _build():
    import concourse.bacc as bacc
    import concourse.mybir as mybir
    import concourse.tile as tile

    FP32 = mybir.dt.float32
    BF16 = mybir.dt.bfloat16
    EXP = mybir.ActivationFunctionType.Exp

    nc = bacc.Bacc("TRN2", target_bir_lowering=False, debug=False,
                   num_devices=NCORES)

    xs = nc.dram_tensor("xs", [TLOC, _D], BF16, kind="ExternalInput")
    xst = nc.dram_tensor("xst", [_D, TLOC], BF16, kind="ExternalInput")
    ms = nc.dram_tensor("ms", [128, TLOC // 128], FP32, kind="ExternalInput")
    wk = nc.dram_tensor("wk", [128, _D], BF16, kind="ExternalInput")
    wq = nc.dram_tensor("wq", [128, _D], BF16, kind="ExternalInput")
    wv = nc.dram_tensor("wv", [128, DC * 512], BF16, kind="ExternalInput")
    op = nc.dram_tensor("op", [128, 4 * _D], BF16, kind="ExternalInput")
    ident = nc.dram_tensor("ident", [128, 128], BF16, kind="ExternalInput")
    ph = nc.dram_tensor("ph", [128, _H], BF16, kind="ExternalInput")
    pht = nc.dram_tensor("pht", [_H, 128], BF16, kind="ExternalInput")
    ones2 = nc.dram_tensor("ones2", [128, 2], BF16, kind="ExternalInput")
    bdmt = nc.dram_tensor("bdmt", [128, 256], FP32, kind="ExternalInput")
    out = nc.dram_tensor("out", [TLOC, _D], BF16, kind="ExternalOutput")

    RG = [[0, 1], [2, 3], [4, 5], [6, 7]]

    with tile.TileContext(nc) as tc:
        with (
            tc.tile_pool(name="const", bufs=1) as cpool,
            tc.tile_pool(name="dram", bufs=1, space="DRAM") as dpool,
        ):
            xst_sb = cpool.tile([128, DC * TLOC], BF16)  # resident X^T
            qst_sb = cpool.tile([128, TLOC], BF16)       # softmax(Q)^T
            eqe_sb = cpool.tile([128, NEARLY * BT], BF16)  # early exp(Q)
            wk_sb = cpool.tile([128, _D], BF16)
            wq_sb = cpool.tile([128, _D], BF16)
            wv_sb = cpool.tile([128, DC * 512], BF16)
            op_sb = cpool.tile([128, 4 * _D], BF16)
            id_sb = cpool.tile([128, 128], BF16)
            ph_sb = cpool.tile([128, _H], BF16)
            pht_sb = cpool.tile([_H, 128], BF16)
            on_sb = cpool.tile([128, 2], BF16)
            ms_sb = cpool.tile([128, TLOC // 128], FP32)
            bdmt_sb = cpool.tile([128, 256], FP32)
            kxp_sb = cpool.tile([128, 1032], BF16)   # packed KX | Ksum
            p1_sb = cpool.tile([64, 1032], BF16)     # reduced partial 1
            p2_sb = cpool.tile([64, 1032], BF16)     # reduced partial 2
            shb_sb = cpool.tile([64, 1032], BF16)    # reduced half state
            sht_sb = cpool.tile([128, DC * 64], BF16)  # KX^T half chunks
            g_sb = cpool.tile([128, _D], BF16)       # gathered G

            aa1_in = dpool.tile([128, 1032], BF16)
            aa1_out = dpool.tile([64, 1032], BF16)
            aa2_in = dpool.tile([128, 1032], BF16)
            aa2_out = dpool.tile([64, 1032], BF16)
            ag_in = dpool.tile([64, _D], BF16)
            ag_out = dpool.tile([128, _D], BF16)
            wdum_in = dpool.tile([128, 2], BF16)
            wdum_out = dpool.tile([64, 2], BF16)

            # ------------- Phase A-K: KX / Ksum state accumulation ---------
            with (
                tc.tile_pool(name="xin", bufs=4) as xin,
                tc.tile_pool(name="esb", bufs=3) as esb,
                tc.tile_pool(name="e2", bufs=8) as e2p,
                tc.tile_pool(name="kt_ps", bufs=2, space="PSUM") as ktp,
                tc.tile_pool(name="scr_ps", bufs=2, space="PSUM") as scr,
                tc.tile_pool(name="kx_ps", bufs=1, space="PSUM") as kxp,
                tc.tile_pool(name="ks_ps", bufs=1, space="PSUM") as ksp,
            ):
                kx_ps = kxp.tile([128, _D], FP32)
                ks_ps = ksp.tile([128, 2], FP32)

                def ktx(k, xts, et):
                    """E-transpose + KX/KS accumulation for block k."""
                    e_ps = scr.tile([128, BT], BF16, tag="scr")
                    for i in range(NT):
                        nc.tensor.transpose(
                            e_ps[:, i * 128:(i + 1) * 128],
                            et[:, i * 128:(i + 1) * 128],
                            id_sb[:],
                        )
                    for i in range(NT):
                        e2 = e2p.tile([128, 128], BF16, tag="e2")
                        j = k * NT + i
                        nc.vector.tensor_scalar_mul(
                            e2[:], e_ps[:, i * 128:(i + 1) * 128],
                            ms_sb[:, j:j + 1],
                        )
                        first = (i == 0 and k == 0)
                        last = (i == NT - 1 and k == NBLK - 1)
                        nc.tensor.matmul(kx_ps[:, 0:512], e2[:],
                                         xts[i][:, 0:512],
                                         start=first, stop=last)
                        nc.tensor.matmul(kx_ps[:, 512:1024], e2[:],
                                         xts[i][:, 512:1024],
                                         start=first, stop=last)
                        nc.tensor.matmul(ks_ps[:], e2[:], on_sb[:],
                                         start=first, stop=last)

                def pack_state():
                    nc.vector.tensor_copy(kxp_sb[:, 0:512], kx_ps[:, 0:512])
                    nc.scalar.copy(kxp_sb[:, 512:1024], kx_ps[:, 512:1024])
                    nc.vector.tensor_copy(kxp_sb[:, 1024:1025],
                                          ks_ps[:, 0:1])
                    nc.vector.memset(kxp_sb[:, 1025:1032], 0.0)

                def load_x(k, xblk):
                    # alternate the two X streams across the two DMA rings
                    # so each block's pair arrives together
                    xb0 = k * DC * BT
                    xe = nc.gpsimd if k % 2 == 0 else nc.sync
                    xo = nc.sync if k % 2 == 0 else nc.gpsimd
                    xe.dma_start(
                        out=xblk[:].rearrange("p (a d) -> p a d", a=NT),
                        in_=xs.ap()[k * BT:(k + 1) * BT, :].rearrange(
                            "(a p) d -> p a d", p=128))
                    xo.dma_start(
                        out=xst_sb[:, xb0:xb0 + DC * BT].rearrange(
                            "p (c t) -> p c t", c=DC),
                        in_=xst.ap().rearrange("(c p) t -> p c t", p=128)
                        [:, :, k * BT:(k + 1) * BT])

                prev = None
                xblk7 = None
                for k in range(NBLK):
                    xb0 = k * DC * BT
                    if k == 0:
                        nc.sync.dma_start(out=wk_sb[:], in_=wk.ap())
                        nc.gpsimd.dma_start(out=id_sb[:], in_=ident.ap())
                        nc.gpsimd.dma_start(out=on_sb[:], in_=ones2.ap())
                        nc.gpsimd.dma_start(out=ms_sb[:], in_=ms.ap())
                        # tiny warm-up collective: absorbs the ncfw
                        # cold-start.  A ReduceScatter specifically -- the
                        # first RS is 15-23us even after an AG warm-up,
                        # while a second RS runs ~8us, so the warm-up must
                        # match the op type of the real state exchange.
                        nc.scalar.dma_start(out=wdum_in[:], in_=ones2.ap())
                        nc.gpsimd.collective_compute(
                            "ReduceScatter",
                            mybir.AluOpType.add,
                            replica_groups=RG,
                            ins=[wdum_in.opt()],
                            outs=[wdum_out.opt()],
                        )
                    if k == 1:
                        nc.sync.dma_start(out=wq_sb[:], in_=wq.ap())

                    if k == NBLK - 1:
                        xblk = xblk7  # prefetched at k == 6
                    else:
                        xblk = xin.tile([128, NT * _D], BF16, tag="xin")
                        load_x(k, xblk)
                    xts = [xblk[:, i * _D:(i + 1) * _D] for i in range(NT)]

                    kt_ps = ktp.tile([128, BT], FP32, tag="kt")
                    for c in range(DC):
                        nc.tensor.matmul(
                            kt_ps[:], wk_sb[:, c * 128:(c + 1) * 128],
                            xst_sb[:, xb0 + c * BT:xb0 + (c + 1) * BT],
                            start=(c == 0), stop=(c == DC - 1),
                        )
                    et = esb.tile([128, BT], BF16, tag="et")
                    nc.scalar.activation(et[:], kt_ps[:], EXP)

                    if prev is not None:
                        ktx(*prev)
                    prev = (k, xts, et)

                    if k == 6:
                        # prefetch block 7's X so nothing can block it in
                        # the ring FIFOs later
                        xblk7 = xin.tile([128, NT * _D], BF16, tag="xin")
                        load_x(NBLK - 1, xblk7)

                    # fill the PE's DMA-starvation holes with early Q-side
                    # projections off already-resident X^T blocks
                    if 2 <= k <= 5:
                        jq = k - 2
                        jb0 = jq * DC * BT
                        qt_ps = ktp.tile([128, BT], FP32, tag="kt")
                        for c in range(DC):
                            nc.tensor.matmul(
                                qt_ps[:], wq_sb[:, c * 128:(c + 1) * 128],
                                xst_sb[:, jb0 + c * BT:jb0 + (c + 1) * BT],
                                start=(c == 0), stop=(c == DC - 1),
                            )
                        nc.scalar.activation(
                            eqe_sb[:, jq * BT:(jq + 1) * BT], qt_ps[:], EXP)
                ktx(*prev)

                # weights for phases B/C load after the X^T stream
                nc.gpsimd.dma_start(out=ph_sb[:], in_=ph.ap())
                nc.gpsimd.dma_start(out=pht_sb[:], in_=pht.ap())
                nc.gpsimd.dma_start(out=bdmt_sb[:], in_=bdmt.ap())
                for c2 in range(DC):
                    nc.gpsimd.dma_start(
                        out=wv_sb[:, c2 * 512:(c2 + 1) * 512],
                        in_=wv.ap()[:, c2 * 512:(c2 + 1) * 512])
                for c2 in range(4):
                    nc.gpsimd.dma_start(
                        out=op_sb[:, c2 * _D:(c2 + 1) * _D],
                        in_=op.ap()[:, c2 * _D:(c2 + 1) * _D])

                pack_state()

            nc.sync.dma_start(out=aa2_in[:], in_=kxp_sb[:])
            nc.gpsimd.collective_compute(
                "ReduceScatter",
                mybir.AluOpType.add,
                replica_groups=RG,
                ins=[aa2_in.opt()],
                outs=[aa2_out.opt()],
            )
            nc.sync.dma_start(out=p2_sb[:], in_=aa2_out[:])

            # ------------- Phase A-Q + B + A-Q2 ---------------------------
            with (
                tc.tile_pool(name="eqsb", bufs=3) as eqsb,
                tc.tile_pool(name="srp", bufs=2) as srp,
                tc.tile_pool(name="bsb1", bufs=1) as bsb1,
                tc.tile_pool(name="qt_ps", bufs=2, space="PSUM") as qtp,
                tc.tile_pool(name="scr2_ps", bufs=3, space="PSUM") as scr2,
                tc.tile_pool(name="kvt_ps", bufs=1, space="PSUM") as kvtp,
                tc.tile_pool(name="g_ps_pool", bufs=1, space="PSUM") as gpp,
            ):
                def qsoft(k, eq):
                    s_ps = scr2.tile([_H, BT], FP32, tag="scr")
                    nc.tensor.matmul(s_ps[:], ph_sb[:], eq[:], start=True,
                                     stop=True)
                    sr = srp.tile([_H, BT], FP32, tag="sr")
                    nc.vector.reciprocal_approx_fast(sr[:], s_ps[:])
                    srb = srp.tile([_H, BT], BF16, tag="srb")
                    nc.vector.tensor_copy(srb[:], sr[:])
                    bq_ps = scr2.tile([128, BT], FP32, tag="scr")
                    nc.tensor.matmul(bq_ps[:], pht_sb[:], srb[:], start=True,
                                     stop=True)
                    nc.vector.tensor_mul(
                        qst_sb[:, k * BT:(k + 1) * BT], eq[:], bq_ps[:]
                    )

                def qphase(blocks):
                    prev = None
                    for k in blocks:
                        if k < NEARLY:
                            eq = eqe_sb[:, k * BT:(k + 1) * BT]
                        else:
                            qt_ps = qtp.tile([128, BT], FP32, tag="qt")
                            xb0 = k * DC * BT
                            for c in range(DC):
                                nc.tensor.matmul(
                                    qt_ps[:],
                                    wq_sb[:, c * 128:(c + 1) * 128],
                                    xst_sb[:, xb0 + c * BT:
                                           xb0 + (c + 1) * BT],
                                    start=(c == 0), stop=(c == DC - 1),
                                )
                            eqt = eqsb.tile([128, BT], BF16, tag="eq")
                            nc.scalar.activation(eqt[:], qt_ps[:], EXP)
                            eq = eqt[:]
                        if prev is not None:
                            qsoft(*prev)
                        prev = (k, eq)
                    qsoft(*prev)

                qphase(range(NBLK - 2))   # blocks 0-5 cover the A2As

                # ---- reduced half state ----------------------------------
                shb_sb = p2_sb
                ksf = bsb1.tile([64, 1], FP32)
                nc.vector.tensor_copy(ksf[:], p2_sb[:, 1024:1025])
                rks = bsb1.tile([64, 1], FP32)
                nc.vector.reciprocal_approx_fast(rks[:], ksf[:])
                # reduced half state as KX^T chunks via xbar transpose
                nc.sync.dma_start_transpose(
                    sht_sb[:].rearrange("p (c l) -> p c l", c=DC),
                    shb_sb[:, 0:1024])

                # ---- Phase B (half, transpose-free) -----------------------
                kvt_ps = kvtp.tile([128, 256], FP32)
                for e in range(4):
                    for c in range(DC):
                        nc.tensor.matmul(
                            kvt_ps[:, e * 64:(e + 1) * 64],
                            wv_sb[:, c * 512 + e * 128:
                                  c * 512 + (e + 1) * 128],
                            sht_sb[:, c * 64:(c + 1) * 64],
                            start=(c == 0), stop=(c == DC - 1),
                        )
                kvbdt = bsb1.tile([128, 256], BF16)
                nc.vector.tensor_mul(kvbdt[:], kvt_ps[:], bdmt_sb[:])

                g_half = bsb1.tile([64, _D], BF16)
                g_ps = gpp.tile([64, 512], FP32, tag="g")
                for e in range(4):
                    nc.tensor.matmul(
                        g_ps[:], kvbdt[:, e * 64:(e + 1) * 64],
                        op_sb[:, e * _D:e * _D + 512],
                        start=(e == 0), stop=(e == 3))
                nc.vector.tensor_scalar_mul(g_half[:, 0:512], g_ps[:],
                                            rks[:])
                g_ps2 = gpp.tile([64, 512], FP32, tag="g")
                for e in range(4):
                    nc.tensor.matmul(
                        g_ps2[:], kvbdt[:, e * 64:(e + 1) * 64],
                        op_sb[:, e * _D + 512:(e + 1) * _D],
                        start=(e == 0), stop=(e == 3))
                nc.vector.tensor_scalar_mul(g_half[:, 512:1024], g_ps2[:],
                                            rks[:])

                # ---- AllGather the two G halves ---------------------------
                nc.sync.dma_start(out=ag_in[:], in_=g_half[:])
                nc.gpsimd.collective_compute(
                    "AllGather",
                    mybir.AluOpType.bypass,
                    replica_groups=RG,
                    ins=[ag_in.opt()],
                    outs=[ag_out.opt()],
                )
                nc.sync.dma_start(out=g_sb[:], in_=ag_out[:])

                # Q blocks 6-7 cover the AllGather latency
                qphase(range(NBLK - 2, NBLK))

            # ------------- Phase C: out = Qs @ G --------------------------
            with (
                tc.tile_pool(name="osb", bufs=3) as osb,
                tc.tile_pool(name="ops", bufs=3, space="PSUM") as ops,
            ):
                for j in range(TLOC // 512):
                    ot = osb.tile([128, 4 * _D], BF16, tag="osb")
                    for h2 in range(4):
                        i = 4 * j + h2
                        o_ps = ops.tile([128, _D], FP32, tag="ops")
                        lhs = qst_sb[:, i * 128:(i + 1) * 128]
                        nc.tensor.matmul(o_ps[:, 0:512], lhs, g_sb[:, 0:512],
                                         start=True, stop=True)
                        nc.tensor.matmul(o_ps[:, 512:1024], lhs,
                                         g_sb[:, 512:1024], start=True,
                                         stop=True)
                        dst = ot[:, h2 * _D:(h2 + 1) * _D]
                        if i % 2 == 0:
                            nc.vector.tensor_copy(dst, o_ps[:])
                        else:
                            nc.scalar.copy(dst, o_ps[:])
                    # alternate output rings (gpsimd is idle in phase C)
                    eng = nc.sync if j % 2 == 0 else nc.gpsimd
                    eng.dma_start(
                        out=out.ap()[j * 512:(j + 1) * 512, :].rearrange(
                            "(a p) d -> p a d", p=128),
                        in_=ot[:].rearrange("p (a d) -> p a d", a=4))

    nc.compile()
    return nc


def _host_inputs(X, attention_mask, Wk, Wq, Wv, o_proj):
    import ml_dtypes

    BF = ml_dtypes.bfloat16
    X = np.asarray(X, dtype=np.float32)
    mask = np.asarray(attention_mask, dtype=np.float32)
    Wk = np.asarray(Wk, dtype=np.float32)
    Wq = np.asarray(Wq, dtype=np.float32)
    Wv = np.asarray(Wv, dtype=np.float32)
    o_proj = np.asarray(o_proj, dtype=np.float32)

    wk_r = np.ascontiguousarray(
        Wk.reshape(DC, 128, _L).transpose(1, 0, 2).reshape(128, DC * _L)
    ).astype(BF)
    wq_r = np.ascontiguousarray(
        Wq.reshape(DC, 128, _L).transpose(1, 0, 2).reshape(128, DC * _L)
    ).astype(BF)
    # per head-half slices of Wv (columns) and o_proj (rows)
    wv_half = []
    op_half = []
    for half in range(2):
        wvh = Wv[:, half * 512:(half + 1) * 512]           # (1024, 512)
        wv_half.append(np.ascontiguousarray(
            wvh.reshape(DC, 128, 512).transpose(1, 0, 2).reshape(
                128, DC * 512)).astype(BF))
        oph = o_proj[half * 512:(half + 1) * 512, :]       # (512, 1024)
        op_half.append(np.ascontiguousarray(
            oph.reshape(4, 128, _D).transpose(1, 0, 2).reshape(
                128, 4 * _D)).astype(BF))
    ident = np.eye(128, dtype=BF)
    ph_m = np.zeros((128, _H), dtype=BF)
    for hh in range(_H):
        ph_m[hh * (_L // _H):(hh + 1) * (_L // _H), hh] = 1.0
    pht_m = np.ascontiguousarray(ph_m.T)
    ones2 = np.ones((128, 2), dtype=BF)
    # transposed block-diag mask: bdmt[p, e*64+l] = 1 iff local d'-row
    # (e*128+p) belongs to the head of local column l
    bdmt_m = np.zeros((128, 256), dtype=np.float32)
    for e in range(4):
        for p in range(128):
            hh = (e * 128 + p) // 64
            bdmt_m[p, e * 64 + hh * 8:e * 64 + (hh + 1) * 8] = 1.0

    Xbf = X.astype(BF)
    in_maps = []
    for core in range(NCORES):
        b, half = core // 2, core % 2
        xsh = np.ascontiguousarray(Xbf[b, half * TLOC:(half + 1) * TLOC, :])
        xsth = np.ascontiguousarray(xsh.T)
        msh = np.ascontiguousarray(
            mask[b, half * TLOC:(half + 1) * TLOC]
            .reshape(TLOC // 128, 128).T)
        in_maps.append({
            "xs": xsh, "xst": xsth, "ms": msh, "wk": wk_r, "wq": wq_r,
            "wv": wv_half[half], "op": op_half[half], "ident": ident,
            "ph": ph_m, "pht": pht_m, "ones2": ones2, "bdmt": bdmt_m,
        })
    return in_maps


def _run(in_maps, trace=False):
    from concourse.bass_utils import run_bass_kernel_spmd

    if "nc" not in _cache:
        _cache["nc"] = _build()
    return run_bass_kernel_spmd(
        _cache["nc"], in_maps, list(range(NCORES)), trace=trace)


def kernel(X, attention_mask, Wk, Wq, Wv, o_proj, n_heads=16):
    in_maps = _host_inputs(X, attention_mask, Wk, Wq, Wv, o_proj)
    res = _run(in_maps)
    out = np.empty((_B, _T, _D), dtype=np.float32)
    for core in range(NCORES):
        b, half = core // 2, core % 2
        out[b, half * TLOC:(half + 1) * TLOC, :] = (
            res.results[core]["out"].astype(np.float32))
    return out
